# revision 10
# baseline (speedup 1.0000x reference)
"""Trainium2 Bass kernel for nn_Net_cora (2-layer GCN + 2WL link predictor).

Algorithmic reformulation (validated against the reference): the dense
(n,n,H) 2WL tensors are never materialized. The output only needs the 2WL
edge state at 2*Q ordered node pairs, and each C[a,b,:] =
sum_k w[a,k,b] * (hA1[a]+hB1[k]+b1) (.) (hA2[k]+hB2[b]+b2) with integer
weights w = cnt(a,k)*cnt(k,b) from the edge index. Expanding the product
turns the 2WL layer into one weighted matmul over nodes plus elementwise
corrections. All floating point math runs on device; the host only builds
integer/structural tables (counts, index vectors, the degree-normalized
aggregation matrix) and re-lays-out weights.

Performance structure:
 - per-core node RELABELING: the ~520 nodes a core touches after the
   second aggregation (pair endpoints + 2WL common neighbors, host-known
   integers) are permuted to the front — 2WL common neighbors first, so
   the pair-weight matmul covers 1 chunk and agg2/projection/gathers 4;
 - A-side and B-side gathers share one PE pass (the ordered-pair list is
   [fwd|rev], so b-side reads are the a-side results with halves
   swapped); b1/b2 biases are added during the PSUM->SBUF copy instead
   of via gather augmentation rows;
 - fp16 feat/S/h tensors and gather one-hots (1 cycle/row on PE, half
   the HBM bytes), fp8e4 pair weights (integer-valued, exact), fp16
   pair-math tail; Wg1@Wg2 folded on host (y = feat@W12 feeds both
   aggregations; aggregation bias terms ride two augmentation rows: S^T
   gains a row-sum row and a ones row);
 - big loads split into sub-DMAs so PE chases the stream; S^T's last
   node chunk ships only its 50 live rows; all small/depend DMAs issue
   early so the serial Sync queue never blocks the tail;
 - the pair-math tail is split across DVE and Pool so its dependency
   chain is ~3 ops deep.

Sharding: the Q=2048 query pairs are split across the 8 cores (256
each); the GCN front-end is replicated on every core. (Collectives
measured ~70-85us first-use on this runtime — replication + dtype
shrink is far cheaper.)
"""

import numpy as np
import ml_dtypes

import concourse.bass as bass
import concourse.mybir as mybir
from concourse import bacc
from concourse.masks import make_identity
from concourse.bass_utils import run_bass_kernel_spmd
from concourse.tile import TileContext

F32 = mybir.dt.float32
F16 = mybir.dt.float16
F8 = mybir.dt.float8e4

N = 1200          # nodes
E = 19200         # edges
H = 20            # hidden dim
F = 1433          # feature dim
FKN = 12          # feature chunks of 128 (last: 25 real rows)
Q = 2048          # query pairs
NCORES = 8
QC = Q // NCORES  # 256 query pairs per core
T = 2 * QC        # 512 ordered pairs per core (forward + reverse)
NCH = 10          # node chunks of 128 (last: 48 real + 2 aug rows)
CHUNKS = [(i * 128, 128) for i in range(9)] + [(1152, 48)]
SLICES = [(0, 512), (512, 1024), (1024, 1200)]  # node free-dim slices
# hcat col blocks (each H wide, 32-aligned): hA1@0 hB2@32 h2@64 (merged
# A/B pass reads 0:96) | hA2@96 hB1@128 pq@160 (W pass reads 96:192)
PCOLS = 160       # projected cols
HC = 192          # hcat cols

_CACHE = {}


def _build_nc(nu, nuw):
    """nu/nuw: 128-node chunks covering the active set / common set."""
    NU = nu * 128
    nc = bacc.Bacc("TRN2", target_bir_lowering=False, debug=False)

    # ------------- DRAM I/O (big tensors host-packed to (128, .)) -------------
    ft_d = [nc.dram_tensor(f"ft_{i}", (128, 11 * (hi - lo)), F16,
                           kind="ExternalInput")
            for i, (lo, hi) in enumerate(SLICES)]
    fttl_d = nc.dram_tensor("fttl", (25, N), F16, kind="ExternalInput")
    w12_d = nc.dram_tensor("w12", (128, FKN * H), F16, kind="ExternalInput")
    st_d = [nc.dram_tensor(f"st_{i}", (128, 9 * (hi - lo)), F16,
                           kind="ExternalInput")
            for i, (lo, hi) in enumerate(SLICES)]
    st9_d = nc.dram_tensor("st9", (50, N), F16, kind="ExternalInput")
    wmt_d = nc.dram_tensor("wmatT", (128, nuw * T), F8, kind="ExternalInput")
    wproj_d = nc.dram_tensor("wproj", (H, PCOLS), F16, kind="ExternalInput")
    c1row_d = nc.dram_tensor("c1row", (1, H), F16, kind="ExternalInput")
    bg2row_d = nc.dram_tensor("bg2row", (1, H), F16, kind="ExternalInput")
    zrow16_d = nc.dram_tensor("zrow16", (1, H), F16, kind="ExternalInput")
    bcol_d = nc.dram_tensor("bcol", (64, 1), F32, kind="ExternalInput")
    w3aug_d = nc.dram_tensor("w3aug", (128, H), F16, kind="ExternalInput")
    wda_d = nc.dram_tensor("wda", (H, 1), F16, kind="ExternalInput")
    wdb_d = nc.dram_tensor("wdb", (H, 1), F16, kind="ExternalInput")
    bd_d = nc.dram_tensor("bd", (1, 1), F32, kind="ExternalInput")
    prow_d = nc.dram_tensor("prow", (1, T), F32, kind="ExternalInput")
    w0srow_d = nc.dram_tensor("w0srow", (1, T), F32, kind="ExternalInput")
    adjrow_d = nc.dram_tensor("adjrow", (1, T), F16, kind="ExternalInput")
    supprow_d = nc.dram_tensor("supprow", (1, T), F16, kind="ExternalInput")
    out_d = nc.dram_tensor("out", (1, QC), F32, kind="ExternalOutput")

    with TileContext(nc) as tc:
        with (
            tc.tile_pool(name="const", bufs=1) as cp,
            tc.tile_pool(name="work", bufs=1) as wp,
            tc.tile_pool(name="loads", bufs=1) as lp,
            tc.tile_pool(name="psum", bufs=8, space="PSUM") as pp,
        ):
            # ---------------- small constants (early, Q14/direct path) -----
            ident16 = cp.tile([128, 128], F16, name="ident16")
            make_identity(nc, ident16)
            iota_t = cp.tile([128, 1], F32, name="iota_t")
            nc.gpsimd.iota(iota_t[:], pattern=[[0, 1]], base=0,
                           channel_multiplier=1,
                           allow_small_or_imprecise_dtypes=True)
            prow_t = cp.tile([1, T], F32, name="prow_t")
            nc.sync.dma_start(out=prow_t[:], in_=prow_d[:])
            w0srow_t = cp.tile([1, T], F32, name="w0srow_t")
            nc.sync.dma_start(out=w0srow_t[:], in_=w0srow_d[:])
            wproj_t = cp.tile([H, PCOLS], F16, name="wproj_t")
            nc.sync.dma_start(out=wproj_t[:], in_=wproj_d[:])
            bcol_t = cp.tile([64, 1], F32, name="bcol_t")
            nc.sync.dma_start(out=bcol_t[:], in_=bcol_d[:])
            w3aug_t = cp.tile([128, H], F16, name="w3aug_t")
            nc.sync.dma_start(out=w3aug_t[:], in_=w3aug_d[:])
            wda_t = cp.tile([H, 1], F16, name="wda_t")
            nc.sync.dma_start(out=wda_t[:], in_=wda_d[:])
            wdb_t = cp.tile([H, 1], F16, name="wdb_t")
            nc.sync.dma_start(out=wdb_t[:], in_=wdb_d[:])
            bd_t = cp.tile([1, 1], F32, name="bd_t")
            nc.sync.dma_start(out=bd_t[:], in_=bd_d[:])

            # ct_big skeleton: zero + adj/supp rows, all off critical path
            ct_big = cp.tile([128, T], F16, name="ct_big")
            nc.gpsimd.memset(ct_big[:].bitcast(F32), 0.0)
            nc.sync.dma_start(out=ct_big[96:97, :], in_=adjrow_d[:])
            nc.sync.dma_start(out=ct_big[97:98, :], in_=supprow_d[:])

            # broadcast index/weight rows (Pool) for the one-hot builds
            p_bc = cp.tile([128, T], F32, name="p_bc")
            nc.gpsimd.partition_broadcast(p_bc[:], prow_t[:])
            w0s_bc = cp.tile([64, T], F32, name="w0s_bc")
            nc.gpsimd.partition_broadcast(w0s_bc[:], w0srow_t[:])

            # merged-pass one-hots built up front (DVE idle in DMA phase):
            # oh[p, t] = (P[t] - p == 128*ci)
            oh_t = []
            for ci in range(nu):
                oh = cp.tile([128, T], F16, name=f"oh_{ci}")
                nc.vector.tensor_scalar(
                    out=oh[:],
                    in0=p_bc[:],
                    scalar1=iota_t[:, 0:1],
                    scalar2=float(ci * 128),
                    op0=mybir.AluOpType.subtract,
                    op1=mybir.AluOpType.is_equal,
                )
                oh_t.append(oh)

            # ---------------- big streaming loads ----------------
            w12t = lp.tile([128, FKN * H], F16, name="w12t")
            nc.sync.dma_start(out=w12t[:], in_=w12_d[:])
            ft_parts, st_parts = [], []
            for i, (lo, hi) in enumerate(SLICES):
                w = hi - lo
                ftp = lp.tile([128, 11 * w], F16, name=f"ft_{i}")
                for c0, c1 in ((0, 4), (4, 8), (8, 11)):
                    nc.sync.dma_start(out=ftp[:, c0 * w:c1 * w],
                                      in_=ft_d[i][:, c0 * w:c1 * w])
                ft_parts.append(ftp)
            ft_tail = lp.tile([25, N], F16, name="ft_tail")
            nc.sync.dma_start(out=ft_tail[:], in_=fttl_d[:])
            for i, (lo, hi) in enumerate(SLICES):
                w = hi - lo
                stp = lp.tile([128, 9 * w], F16, name=f"st_{i}")
                for c0, c1 in ((0, 5), (5, 9)):
                    nc.sync.dma_start(out=stp[:, c0 * w:c1 * w],
                                      in_=st_d[i][:, c0 * w:c1 * w])
                st_parts.append(stp)
            st9_t = lp.tile([50, N], F16, name="st9_t")
            nc.sync.dma_start(out=st9_t[:], in_=st9_d[:])
            wmt = lp.tile([128, nuw * T], F8, name="wmt")
            nc.sync.dma_start(out=wmt[:], in_=wmt_d[:])

            # ------------- yT = (feat @ W12)^T  (20, 1200) -------------
            y16T = wp.tile([H, N], F16, name="y16T")
            for si, (lo, hi) in enumerate(SLICES):
                w = hi - lo
                pz = pp.tile([H, w], F32, name="pz", tag="ps")
                for ki in range(FKN):
                    if ki == 11:
                        rows, rhs = 25, ft_tail[:, lo:hi]
                    else:
                        rows = 128
                        rhs = ft_parts[si][:, ki * w:(ki + 1) * w]
                    nc.tensor.matmul(
                        pz[:], w12t[:rows, ki * H:(ki + 1) * H], rhs,
                        start=(ki == 0), stop=(ki == FKN - 1))
                nc.vector.tensor_copy(out=y16T[:, lo:hi], in_=pz[:])

            # transpose (20, n)-slices into 128-node chunks with two aug
            # rows on the last chunk (partitions 48/49 pair with the s1s
            # and ones rows of S^T_aug); aug DMAs issue before the copies
            def transpose_to_chunks(srcT, aug48, aug49, label):
                zall = wp.tile([128, NCH * H], F16, name=f"z{label}all")
                tl = zall[:, (NCH - 1) * H:NCH * H]
                nc.sync.dma_start(out=tl[48:49, :], in_=aug48[:])
                nc.sync.dma_start(out=tl[49:50, :], in_=aug49[:])
                ptall = pp.tile([128, NCH * H], F16, name=f"pt_{label}",
                                tag="ps")
                for ci, (off, cnt) in enumerate(CHUNKS):
                    nc.tensor.transpose(
                        ptall[:cnt, ci * H:(ci + 1) * H],
                        srcT[:, off:off + cnt], ident16[:H, :H])
                nc.vector.tensor_copy(
                    out=zall[:, :(NCH - 1) * H], in_=ptall[:, :(NCH - 1) * H])
                nc.vector.tensor_copy(
                    out=zall[:48, (NCH - 1) * H:],
                    in_=ptall[:48, (NCH - 1) * H:])
                return [zall[:, ci * H:(ci + 1) * H] for ci in range(NCH)]

            y_t = transpose_to_chunks(y16T, zrow16_d, c1row_d, "y")

            # ------------- two aggregations hT = (S_aug @ z_aug)^T -------------
            def aggregate(z_tiles, outT, ncols):
                for lo, hi in SLICES:
                    if lo >= ncols:
                        break
                    hi = min(hi, ncols)
                    si, w = lo // 512, SLICES[lo // 512][1] - lo
                    ph = pp.tile([H, hi - lo], F32, name="ph", tag="ps")
                    for ci in range(NCH):
                        if ci < 9:
                            rows = 128
                            rhs = st_parts[si][:, ci * w: ci * w + hi - lo]
                        else:
                            rows = 50
                            rhs = st9_t[:50, lo:hi]
                        nc.tensor.matmul(
                            ph[:], z_tiles[ci][:rows, :], rhs,
                            start=(ci == 0), stop=(ci == NCH - 1))
                    nc.vector.tensor_copy(out=outT[:, lo:hi], in_=ph[:])

            t16T = wp.tile([H, N], F16, name="t16T")
            aggregate(y_t, t16T, N)
            t_t = transpose_to_chunks(t16T, c1row_d, bg2row_d, "t")

            h2T = wp.tile([H, NU], F16, name="h2T")
            aggregate(t_t, h2T, NU)

            # ------------- projections -> hcat (active-node chunks) -------------
            hcat_t = []
            for ci in range(nu):
                ppx = pp.tile([128, PCOLS], F32, name="ppx", tag="ps")
                nc.tensor.matmul(
                    ppx[:], h2T[:, ci * 128:(ci + 1) * 128], wproj_t[:],
                    start=True, stop=True)
                hc = wp.tile([128, HC], F16, name=f"hcat_{ci}")
                nc.gpsimd.memset(hc[:, 180:192].bitcast(F32), 0.0)
                nc.scalar.copy(out=hc[:, 0:PCOLS], in_=ppx[:])
                # pq = hA2 * hB1
                nc.vector.tensor_mul(
                    out=hc[:, 160:180], in0=hc[:, 96:116], in1=hc[:, 128:148])
                hcat_t.append(hc)

            # ------------- merged A/B gather + W pass -------------
            psM = pp.tile([96, T], F32, name="psM", tag="ps")
            for ci in range(nu):
                nc.tensor.matmul(
                    psM[:], hcat_t[ci][:, 0:96], oh_t[ci][:],
                    start=(ci == 0), stop=(ci == nu - 1))
            # combM rows: hA1[a]+b1 @0, hB2[a]+b2 @32, h2[a] @64
            combM = wp.tile([96, T], F32, name="combM")
            nc.vector.tensor_scalar_add(combM[0:H, :], psM[0:H, :],
                                        bcol_t[0:H, 0:1])
            nc.vector.tensor_scalar_add(combM[32:32 + H, :],
                                        psM[32:32 + H, :],
                                        bcol_t[32:32 + H, 0:1])
            nc.scalar.copy(out=combM[64:64 + H, :], in_=psM[64:64 + H, :])
            A1 = combM[0:H, :]
            B2g = combM[32:32 + H, :]   # b-side reads swap the halves
            Mh2 = combM[64:64 + H, :]

            psW = pp.tile([96, T], F32, name="psW", tag="ps")
            for ci in range(nuw):
                nc.tensor.matmul(
                    psW[:], hcat_t[ci][:, 96:192], wmt[:, ci * T:(ci + 1) * T],
                    start=(ci == 0), stop=(ci == nuw - 1))
            WQ = psW[0:H, :]
            WP = psW[32:32 + H, :]
            WPQ = psW[64:64 + H, :]

            # ------------- pair math (T-orient, 20 x 512 tiles) -------------
            # supp*C = u*(w0s*v + WQ) + v*WP + WPQ; additive terms live in
            # separate 32-aligned row blocks of ct_big and the X1 matmul's
            # stationary operand replicates W3h across them so the PE
            # contraction performs the adds for free. DVE/Pool split keeps
            # the chain ~3 ops deep.
            zxx = wp.tile([H, QC], F16, name="zxx")
            nc.gpsimd.tensor_mul(out=zxx[:], in0=Mh2[:, 0:QC],
                                 in1=Mh2[:, QC:T])
            nc.scalar.copy(out=ct_big[64:64 + H, :], in_=WPQ)
            vw = wp.tile([64, T], F32, name="vw")
            nc.vector.tensor_mul(out=vw[32:32 + H, 0:QC], in0=B2g[:, QC:T],
                                 in1=w0s_bc[32:32 + H, 0:QC])
            nc.gpsimd.tensor_mul(out=vw[32:32 + H, QC:T], in0=B2g[:, 0:QC],
                                 in1=w0s_bc[32:32 + H, QC:T])
            s1 = wp.tile([H, T], F32, name="s1")
            nc.vector.tensor_add(out=s1[:], in0=vw[32:32 + H, :], in1=WQ)
            nc.vector.tensor_mul(out=ct_big[0:H, :], in0=A1, in1=s1[:])
            nc.vector.tensor_mul(out=ct_big[32:32 + H, 0:QC],
                                 in0=B2g[:, QC:T], in1=WP[:, 0:QC])
            nc.vector.tensor_mul(out=ct_big[32:32 + H, QC:T],
                                 in0=B2g[:, 0:QC], in1=WP[:, QC:T])

            # X1T = W3big.T @ ct_big  (20, 512): w3aug rows are
            # [W3h@0 | W3h@32 | W3h@64 | w3a@96 | b3@97]
            x1T = pp.tile([H, T], F32, name="x1T", tag="ps")
            nc.tensor.matmul(x1T[:], w3aug_t[:], ct_big[:],
                             start=True, stop=True)
            x1s = wp.tile([H, QC], F32, name="x1s")
            nc.scalar.copy(out=x1s[:], in_=x1T[:, QC:T])

            # xpT = X1T[:, :QC]*X1T[:, QC:]
            zxp = wp.tile([H, QC], F16, name="zxp")
            nc.vector.tensor_mul(out=zxp[:], in0=x1T[:, 0:QC], in1=x1s[:])

            # out = WdA.T @ xpT + WdB.T @ xxT + bd  (1, 256)
            oxp = pp.tile([1, QC], F32, name="oxp", tag="ps")
            nc.tensor.matmul(oxp[:], wda_t[:], zxp[:], start=True, stop=False)
            nc.tensor.matmul(oxp[:], wdb_t[:], zxx[:], start=False, stop=True)
            orow = wp.tile([1, QC], F32, name="orow")
            nc.vector.tensor_scalar_add(orow[:], oxp[:], bd_t[:, 0:1])
            nc.sync.dma_start(out=out_d[:], in_=orow[:])

    nc.compile()
    return nc


def _host_prep(inputs):
    """Pure index/structural preprocessing + weight re-layout. Returns the
    per-core input maps and the chunk counts for the active node set."""
    ei = np.asarray(inputs["ei"], np.int64)
    pos1 = np.asarray(inputs["pos1"], np.int64)
    pos2 = np.asarray(inputs["pos2"], np.int64)
    feat = np.asarray(inputs["feat"], np.float32)
    Wg1 = np.asarray(inputs["Wg1"], np.float32)
    bg1 = np.asarray(inputs["bg1"], np.float32)
    Wg2 = np.asarray(inputs["Wg2"], np.float32)
    bg2 = np.asarray(inputs["bg2"], np.float32)
    W1 = np.asarray(inputs["W1"], np.float32)
    b1 = np.asarray(inputs["b1"], np.float32)
    W2 = np.asarray(inputs["W2"], np.float32)
    b2 = np.asarray(inputs["b2"], np.float32)
    W3 = np.asarray(inputs["W3"], np.float32)
    b3 = np.asarray(inputs["b3"], np.float32)
    Wd = np.asarray(inputs["Wd"], np.float32)
    bd = np.asarray(inputs["bd"], np.float32)

    src, dst = ei[0], ei[1]
    pos = pos1[pos2][:, 0].reshape(-1, 2)  # (Q, 2)

    # structural tables (integers only)
    cnt = np.zeros((N, N), np.float32)
    np.add.at(cnt, (src, dst), 1.0)
    deg = np.zeros((N,), np.float64)
    np.add.at(deg, dst, 1.0)
    deg += 1.0
    dinv = (deg ** -0.5).astype(np.float32)
    S = (dinv[:, None] * dinv[None, :]) * cnt.T
    S[np.arange(N), np.arange(N)] += dinv * dinv

    # weight re-layout (host does only O(F*H) weight math)
    W12 = (Wg1 @ Wg2).astype(np.float32)
    c1 = (bg1 @ Wg2).astype(np.float32)
    w12_pad = np.zeros((FKN * 128, H), np.float32)
    w12_pad[:F] = W12
    wdb = Wd[H:2 * H, 0]
    wproj = np.zeros((H, PCOLS), np.float32)
    for off, blk in zip(
        (0, 32, 64, 96, 128),
        (W1[:H], W2[H:], np.eye(H, dtype=np.float32), W2[:H], W1[H:]),
    ):
        wproj[:, off:off + H] = blk
    w3aug = np.zeros((128, H), np.float32)
    w3aug[0:H] = W3[:H]
    w3aug[32:32 + H] = W3[:H]
    w3aug[64:64 + H] = W3[:H]
    w3aug[96] = W3[H]
    w3aug[97] = b3
    bcol = np.zeros((64, 1), np.float32)
    bcol[0:H, 0] = b1
    bcol[32:32 + H, 0] = b2

    shared = {
        "w12": np.ascontiguousarray(
            w12_pad.reshape(FKN, 128, H).transpose(1, 0, 2).reshape(128, -1)
        ).astype(np.float16),
        "wproj": wproj.astype(np.float16),
        "c1row": c1.reshape(1, H).astype(np.float16),
        "bg2row": bg2.reshape(1, H).astype(np.float16),
        "zrow16": np.zeros((1, H), np.float16),
        "bcol": bcol,
        "w3aug": w3aug.astype(np.float16),
        "wda": Wd[:H].reshape(H, 1).astype(np.float16),
        "wdb": wdb.reshape(H, 1).astype(np.float16),
        "bd": bd.reshape(1, 1),
    }

    # per-core active node sets: 2WL common neighbors first, then the
    # remaining pair endpoints
    percore = []
    nu = nuw = 0
    for c in range(NCORES):
        qs = slice(c * QC, (c + 1) * QC)
        a = np.concatenate([pos[qs, 0], pos[qs, 1]])  # (T,)
        b = np.concatenate([pos[qs, 1], pos[qs, 0]])
        wmat = cnt[a, :] * cnt[:, b].T  # (T, N) integer-valued
        ks = np.nonzero(wmat.any(axis=0))[0]
        endp = np.setdiff1d(np.unique(np.concatenate([a, b])), ks,
                            assume_unique=False)
        u = np.concatenate([ks, endp])
        percore.append((a, b, wmat, u, len(ks)))
        nu = max(nu, (len(u) + 127) // 128)
        nuw = max(nuw, 1, (len(ks) + 127) // 128)

    in_maps = []
    for c in range(NCORES):
        a, b, wmat, u, nk = percore[c]
        NU = nu * 128
        rest = np.setdiff1d(np.arange(N), u, assume_unique=False)
        perm = np.concatenate([u, rest])
        inv = np.empty(N, np.int64)
        inv[perm] = np.arange(N)
        an = inv[a]
        assert an.max() < NU and inv[b].max() < NU

        featP = feat[perm]
        SP = S[perm][:, perm]
        featT_pad = np.zeros((FKN * 128, N), np.float16)
        featT_pad[:F] = featP.T.astype(np.float16)
        st_pad = np.zeros((9 * 128 + 50, N), np.float32)
        st_pad[:N] = SP.T
        st_pad[N] = SP.sum(axis=1)   # pairs with c1/bg2 aug rows
        st_pad[N + 1] = 1.0          # pairs with the plain bias rows

        w0 = wmat.sum(1)
        adjv = (cnt[a, b] > 0).astype(np.float32)
        suppv = ((w0 > 0) | (adjv > 0)).astype(np.float32)
        w0s = (w0 * suppv).astype(np.float32)
        # pair-weight rows in new labels: nonzero rows all sit in the
        # common-neighbor prefix
        wmU = (wmat.T * suppv[None, :])[perm[:nuw * 128]]
        wm8 = wmU.astype(ml_dtypes.float8_e4m3)
        assert np.array_equal(wm8.astype(np.float32), wmU), \
            "pair weights not exact in fp8e4"

        m = dict(shared)
        fchunks = featT_pad[:1408].reshape(11, 128, N)
        schunks = st_pad[:1152].astype(np.float16).reshape(9, 128, N)
        for i, (lo, hi) in enumerate(SLICES):
            m[f"ft_{i}"] = np.ascontiguousarray(
                fchunks[:, :, lo:hi].transpose(1, 0, 2).reshape(128, -1))
            m[f"st_{i}"] = np.ascontiguousarray(
                schunks[:, :, lo:hi].transpose(1, 0, 2).reshape(128, -1))
        m["fttl"] = np.ascontiguousarray(featT_pad[1408:1433])
        m["st9"] = np.ascontiguousarray(st_pad[1152:1202]).astype(np.float16)
        m["wmatT"] = np.ascontiguousarray(
            wm8.reshape(nuw, 128, T).transpose(1, 0, 2).reshape(128, -1))
        m["prow"] = an.astype(np.float32).reshape(1, T)
        m["w0srow"] = w0s.reshape(1, T)
        m["adjrow"] = adjv.reshape(1, T).astype(np.float16)
        m["supprow"] = suppv.reshape(1, T).astype(np.float16)
        in_maps.append(m)
    return in_maps, nu, nuw


def kernel(**inputs):
    in_maps, nu, nuw = _host_prep(inputs)
    key = ("nc", nu, nuw)
    if key not in _CACHE:
        _CACHE[key] = _build_nc(nu, nuw)
    nc = _CACHE[key]
    res = run_bass_kernel_spmd(nc, in_maps, core_ids=list(range(NCORES)))
    outs = [res.results[c]["out"].reshape(QC, 1) for c in range(NCORES)]
    return np.concatenate(outs, 0).astype(np.float32)
